# revision 9
# baseline (speedup 1.0000x reference)
"""Trainium2 Bass kernel for nn_Explainer segment_reduce (cdist + bidirectional
segment max/mean) on 8 NeuronCores.

Math (reference):
    ef_n = (h[ne0] + h[ne1])/2, ef_l = (h[le0] + h[le1])/2
    M = -cdist(ef_n, ef_l)                      # [En, El]
    out_n = seg_mean_rows(seg_max_cols(M))      # [Gn, Gl]
    out_l = seg_mean_cols(seg_max_rows(M))      # [Gn, Gl]
    out = (out_n + out_l)/2

Device strategy (per core c of 8):
  - Core c owns node segments [8c, 8c+8). Its rows are laid out in "bands":
    local segment s -> lanes [16s, 16s+16), row-tiles t in [0, nrt).
  - Label edges are replicated, padded per segment to LW columns (duplicated
    edges; duplicates can't change a min and are weight-masked out of sums).
  - With u = h[e0]+h[e1] (so ef = u/2): d^2 = 0.25*(|u_n|^2 + |u_l|^2 - 2 u_n.u_l).
    PE computes psum = |u_l|^2 - 2 u_n.u_l via K=257 matmuls (two K=128 feature
    chunks of -2*u_nT, plus a K=1 ones x bl2 row). ACT then emits
    e = -(0.25*psum + 0.25*|u_n|^2 + BIG*dummy_mask) in fp16 -> SBUF.
    So e = -0.25*d^2 (negated): every min becomes a max (HW has no Pool-min).
  - Row side: one 3D-AP segmented reduce(max) per row-tile -> e_rowmax [128, 64].
  - Col side: running tensor_tensor(max) into coll [128, cols]; then a 32x32
    block transpose + 8 banded reduces collapse each 16-lane band (= one node
    segment) -> e_colmax.
  - Host: sq = -4*e, clamp, sqrt, masked means, assemble [64, 64].
"""
import numpy as np

import concourse.bacc as bacc
import concourse.tile as tile
import concourse.mybir as mybir
from concourse.alu_op_type import AluOpType
from concourse.masks import make_identity
from concourse.bass_utils import run_bass_kernel_spmd

P = 128
N_CORES = 8
GN = GL = 64
D = 256
BIG = 1.0e4
F16 = mybir.dt.float16
F32 = mybir.dt.float32
I16 = mybir.dt.int16

_prog_cache = {}


def _wrap_idx(idx: np.ndarray) -> np.ndarray:
    """dma_gather index layout: [128, n/16] int16; idx i at [i%16, i//16], x8 groups."""
    n = idx.shape[0]
    base = np.ascontiguousarray(idx.reshape(n // 16, 16).T).astype(np.int16)
    return np.ascontiguousarray(np.tile(base, (8, 1)))


def _build(nrt: int, cols: int, n_h: int, lw: int, level: int = 99):
    NG = cols // 2048          # column groups (psum tiles of 2048)
    NCH = cols // 2048         # label gather chunks of 2048 idxs
    bw = cols // 32            # 32-blocks per row

    nc = bacc.Bacc("TRN2", target_bir_lowering=False, debug=False,
                   num_devices=N_CORES)
    hbf = nc.dram_tensor("hbf", [n_h, D], F16, kind="ExternalInput")
    idxn0 = nc.dram_tensor("idxn0", [P, nrt * 8], I16, kind="ExternalInput")
    idxn1 = nc.dram_tensor("idxn1", [P, nrt * 8], I16, kind="ExternalInput")
    idxl0 = nc.dram_tensor("idxl0", [P, cols // 16], I16, kind="ExternalInput")
    idxl1 = nc.dram_tensor("idxl1", [P, cols // 16], I16, kind="ExternalInput")
    maskq = nc.dram_tensor("maskq", [P, nrt], F32, kind="ExternalInput")
    rowout = nc.dram_tensor("rowout", [P, nrt * GL], F16, kind="ExternalOutput")
    bandout = nc.dram_tensor("bandout", [P, 2 * bw], F16, kind="ExternalOutput")

    with tile.TileContext(nc) as tc:
        with (
            tc.tile_pool(name="persist", bufs=1) as pp,
            tc.tile_pool(name="strip", bufs=2) as sp,
            tc.tile_pool(name="gbuf", bufs=2) as gp,
            tc.tile_pool(name="small", bufs=2) as smp,
        ):
            u_lT = pp.tile([P, 2, cols], F16, tag="u_lT")
            bl2v = pp.tile([1, cols], F16, tag="bl2v")
            u_nT = pp.tile([P, nrt * 2, P], F16, tag="u_nT")
            an2q = pp.tile([P, nrt], F32, tag="an2q")
            coll = pp.tile([P, cols], F16, tag="coll")
            ones_col = pp.tile([P, 1], F16, tag="ones_col")
            ones_row = pp.tile([1, P], F16, tag="ones_row")
            ident = pp.tile([P, P], F16, tag="ident")
            it_n0 = pp.tile([P, nrt * 8], I16, tag="it_n0")
            it_n1 = pp.tile([P, nrt * 8], I16, tag="it_n1")
            it_l0 = pp.tile([P, cols // 16], I16, tag="it_l0")
            it_l1 = pp.tile([P, cols // 16], I16, tag="it_l1")
            maskt = pp.tile([P, nrt], F32, tag="maskt")
            gn0 = pp.tile([P, nrt, D], F16, tag="gn0")
            gn1 = pp.tile([P, nrt, D], F16, tag="gn1")
            u_n = pp.tile([P, nrt, D], F16, tag="u_n")
            band_t = pp.tile([P, 2 * bw], F16, tag="band_t")

            nc.gpsimd.memset(ones_col[:], 1.0)
            nc.gpsimd.memset(ones_row[:], 1.0)
            nc.gpsimd.memset(coll[:], -BIG)
            make_identity(nc, ident[:])

            nc.sync.dma_start(it_n0[:], idxn0[:])
            nc.sync.dma_start(it_n1[:], idxn1[:])
            nc.sync.dma_start(it_l0[:], idxl0[:])
            nc.sync.dma_start(it_l1[:], idxl1[:])
            nc.sync.dma_start(maskt[:], maskq[:])

            with tc.tile_pool(name="preps", bufs=2, space="PSUM") as preps:
                # ---- node side ----
                if level >= 1:
                    nc.gpsimd.dma_gather(gn0[:], hbf[:], it_n0[:], nrt * P, nrt * P, D, single_packet=False)
                    nc.gpsimd.dma_gather(gn1[:], hbf[:], it_n1[:], nrt * P, nrt * P, D, single_packet=False)
                    nc.vector.tensor_add(u_n[:], gn0[:], gn1[:])
                for t in range(nrt if level >= 4 else 0):
                    scratch = smp.tile([P, D], F16, tag="scratch")
                    nc.vector.affine_mul_reduce(
                        out=scratch[:], accum_out=an2q[:, t:t + 1],
                        in0=u_n[:, t, :], in1=u_n[:, t, :],
                        scale=-0.25, bias=0.0,
                    )
                if level >= 4:
                    nc.vector.tensor_add(an2q[:], an2q[:], maskt[:])
                for t in range(nrt if level >= 4 else 0):
                    for k in range(2):
                        ptr = preps.tile([P, P], F16, tag="tr")
                        nc.tensor.transpose(
                            ptr[:], u_n[:, t, k * P:(k + 1) * P], ident[:])
                        nc.vector.tensor_scalar_mul(
                            u_nT[:, 2 * t + k, :], ptr[:], -2.0)

                # ---- label side ----
                for c in range(NCH if level >= 2 else 0):
                    ga = gp.tile([P, 2, 2048], F16, tag="ga")
                    gb = gp.tile([P, 2, 2048], F16, tag="gb")
                    nc.gpsimd.dma_gather(
                        ga[:], hbf[:], it_l0[:, c * 128:(c + 1) * 128],
                        2048, 2048, D, transpose=True, single_packet=False)
                    nc.gpsimd.dma_gather(
                        gb[:], hbf[:], it_l1[:, c * 128:(c + 1) * 128],
                        2048, 2048, D, transpose=True, single_packet=False)
                    for k in range(2):
                        nc.vector.tensor_add(
                            u_lT[:, k, c * 2048:(c + 1) * 2048],
                            ga[:, k, :], gb[:, k, :])

                # ---- bl2 = |u_l|^2 via squares + ones-matmul partition sum ----
                sq0 = sp.tile([P, cols], F16, tag="strip")
                sq1 = sp.tile([P, cols], F16, tag="strip")
                if level >= 3:
                    nc.gpsimd.tensor_mul(sq0[:], u_lT[:, 0, :], u_lT[:, 0, :])
                    nc.gpsimd.tensor_mul(sq1[:], u_lT[:, 1, :], u_lT[:, 1, :])
                for g2 in range(cols // 512 if level >= 3 else 0):
                    pbl = preps.tile([1, 512], F32, tag="bl")
                    sl = slice(g2 * 512, (g2 + 1) * 512)
                    nc.tensor.matmul(pbl[:], ones_col[:], sq0[:, sl],
                                     start=True, stop=False)
                    nc.tensor.matmul(pbl[:], ones_col[:], sq1[:, sl],
                                     start=False, stop=True)
                    nc.scalar.activation(
                        bl2v[:, sl], pbl[:],
                        mybir.ActivationFunctionType.Copy)

            # ---- main loop ----
            with tc.tile_pool(name="ps", bufs=2, space="PSUM") as ps:
                for t in range(0 if level < 5 else (1 if level == 5 else (2 if level == 6 else nrt))):
                    strip = sp.tile([P, cols], F16, tag="strip")
                    for g in range(NG):
                        pt = ps.tile([P, 2048], F32, tag="dot")
                        for s in range(4):
                            osl = slice(s * 512, (s + 1) * 512)
                            csl = slice(g * 2048 + s * 512, g * 2048 + (s + 1) * 512)
                            nc.tensor.matmul(pt[:, osl], u_nT[:, 2 * t, :],
                                             u_lT[:, 0, csl], start=True, stop=False)
                        for s in range(4):
                            osl = slice(s * 512, (s + 1) * 512)
                            csl = slice(g * 2048 + s * 512, g * 2048 + (s + 1) * 512)
                            nc.tensor.matmul(pt[:, osl], u_nT[:, 2 * t + 1, :],
                                             u_lT[:, 1, csl], start=False, stop=False)
                        for s in range(4):
                            osl = slice(s * 512, (s + 1) * 512)
                            csl = slice(g * 2048 + s * 512, g * 2048 + (s + 1) * 512)
                            nc.tensor.matmul(pt[:, osl], ones_row[:],
                                             bl2v[:, csl], start=False, stop=True)
                        nc.scalar.activation(
                            strip[:, g * 2048:(g + 1) * 2048], pt[:],
                            mybir.ActivationFunctionType.Identity,
                            bias=an2q[:, t:t + 1], scale=-0.25)
                    rmx = smp.tile([P, GL], F16, tag="rowmax")
                    nc.vector.tensor_reduce(
                        rmx[:], strip[:].rearrange("p (s w) -> p s w", w=lw),
                        axis=mybir.AxisListType.X, op=AluOpType.max)
                    nc.sync.dma_start(rowout[:, t * GL:(t + 1) * GL], rmx[:])
                    nc.vector.tensor_max(coll[:], coll[:], strip[:])

                # ---- collapse 16-lane bands -> per-segment col maxes ----
                if level >= 8:
                    collT = sp.tile([P, cols], F16, tag="strip")
                    nc.vector.transpose(collT[:], coll[:])
                    for b in range(8 if level >= 9 else 0):
                        pb = 32 * (b // 2)
                        m = b % 2
                        nc.vector.tensor_reduce(
                            band_t[pb:pb + 32, m * bw:(m + 1) * bw],
                            collT[pb:pb + 32, :]
                            .rearrange("p (j q) -> p j q", q=32)[:, :, 16 * m:16 * m + 16],
                            axis=mybir.AxisListType.X, op=AluOpType.max)
                    if level >= 10:
                        nc.sync.dma_start(bandout[:], band_t[:])

    nc.compile()
    return nc


def _get_program(nrt, cols, n_h, lw, level=99):
    key = (nrt, cols, n_h, lw, level)
    if key not in _prog_cache:
        _prog_cache[key] = _build(nrt, cols, n_h, lw, level)
    return _prog_cache[key]


def kernel(h, node_edge, node_batch, label_edge, label_batch):
    h = np.asarray(h)
    ne = np.asarray(node_edge).astype(np.int64)
    nb = np.asarray(node_batch).astype(np.int64)
    le = np.asarray(label_edge).astype(np.int64)
    lb = np.asarray(label_batch).astype(np.int64)
    n_h = h.shape[0]

    cn = np.bincount(nb, minlength=GN).astype(np.int64)
    cl = np.bincount(lb, minlength=GL).astype(np.int64)
    nb_off = np.concatenate([[0], np.cumsum(cn)])
    lb_off = np.concatenate([[0], np.cumsum(cl)])

    lw = 160
    while cl.max() > lw:
        lw += 32
    cols = GL * lw
    nrt = max(1, int(-(-cn.max() // 16)))
    nrows = nrt * P
    bw = cols // 32

    # label column layout (shared by all cores)
    k_in_seg = np.tile(np.arange(lw), GL)
    seg_of_col = np.repeat(np.arange(GL), lw)
    sz = cl[seg_of_col]
    col_w = (k_in_seg < sz).astype(np.float64)
    safe_sz = np.maximum(sz, 1)
    col_edge = lb_off[seg_of_col] + (k_in_seg % safe_sz)
    col_edge = np.where(sz > 0, col_edge, 0)
    idxl0 = _wrap_idx(le[0][col_edge].astype(np.int16))
    idxl1 = _wrap_idx(le[1][col_edge].astype(np.int16))

    hf16 = np.ascontiguousarray(h.astype(np.float16))

    in_maps = []
    for c in range(N_CORES):
        slot = np.full(nrows, -1, np.int64)
        for s in range(8):
            g = 8 * c + s
            rows = np.arange(nb_off[g], nb_off[g + 1])
            j = np.arange(rows.size)
            slot[(j // 16) * P + 16 * s + (j % 16)] = rows
        rows_safe = np.where(slot < 0, 0, slot)
        mq = np.where(slot < 0, -BIG, 0.0).astype(np.float32)
        in_maps.append({
            "hbf": hf16,
            "idxn0": _wrap_idx(ne[0][rows_safe].astype(np.int16)),
            "idxn1": _wrap_idx(ne[1][rows_safe].astype(np.int16)),
            "idxl0": idxl0,
            "idxl1": idxl1,
            # slot m -> [m % 128, m // 128]
            "maskq": np.ascontiguousarray(mq.reshape(nrt, P).T),
        })

    nc = _get_program(nrt, cols, n_h, lw)
    res = run_bass_kernel_spmd(nc, in_maps, core_ids=list(range(N_CORES)))

    out_n = np.zeros((GN, GL), np.float64)
    out_l = np.zeros((GN, GL), np.float64)
    for c in range(N_CORES):
        r = res.results[c]
        rowe = r["rowout"].astype(np.float64).reshape(P, nrt, GL)
        bande = r["bandout"].astype(np.float64)
        for s in range(8):
            g = 8 * c + s
            if cn[g] == 0:
                continue
            j = np.arange(cn[g])
            lanes = 16 * s + (j % 16)
            ts = j // 16
            ev = rowe[lanes, ts, :]                       # [seg_rows, GL]
            dmin = np.sqrt(np.maximum(-ev, 0.0))
            row_mean = -dmin.mean(axis=0)
            row_mean[cl == 0] = 0.0
            out_n[g] = row_mean

            i2, m = divmod(s, 2)
            block = bande[32 * i2:32 * i2 + 32, m * bw:(m + 1) * bw]  # [32, bw]
            ecol = block.T.reshape(cols)                  # F = 32*j + q'
            dcol = np.sqrt(np.maximum(-ecol, 0.0))
            sums = (dcol * col_w).reshape(GL, lw).sum(axis=1)
            col_mean = -(sums / np.maximum(cl, 1))
            col_mean[cl == 0] = 0.0
            out_l[g] = col_mean

    return ((out_n + out_l) * 0.5).astype(np.float32)


# revision 10
# speedup vs baseline: 2.4543x; 2.4543x over previous
"""Trainium2 Bass kernel for nn_Explainer segment_reduce (cdist + bidirectional
segment max/mean) on 8 NeuronCores.

Math (reference):
    ef_n = (h[ne0] + h[ne1])/2, ef_l = (h[le0] + h[le1])/2
    M = -cdist(ef_n, ef_l)                      # [En, El]
    out_n = seg_mean_rows(seg_max_cols(M))      # [Gn, Gl]
    out_l = seg_mean_cols(seg_max_rows(M))      # [Gn, Gl]
    out = (out_n + out_l)/2

Sharding: core c owns node segments [8c, 8c+8) (data parallel over node
edges); label-edge features are replicated (host-computed u_l = h[le0]+h[le1],
padded per label segment to LW columns with duplicated edges; dups can't
change a segment min and are weight-masked out of the segment mean).

Per core, with u = h[e0]+h[e1] (ef = u/2): d^2 = 0.25*(|u_n|^2+|u_l|^2-2u_n.u_l).
  - Node rows live in per-segment lane bands: segment s -> lanes
    [B_s, B_s+L_s), row-tiles t in [0, nrt). Dummy slots get -BIG bias.
  - PE: psum = |u_l|^2 - 2 u_n.u_l via K=257 matmuls (2 K=128 chunks of
    -2*u_nT gathered+transposed on device, plus a K=1 ones x bl2 row).
  - ACT: e = -(0.25*psum) - 0.25*|u_n|^2 (- BIG if dummy) -> fp16 strip.
    e == -d^2 exactly; negation turns all mins into maxes.
  - DVE row side: 5-way strided TT-max tree + 3D reduce -> e_rowmax [128,64].
  - DVE col side: running TT-max into coll [128, cols].
Host: band-collapse coll, clamp/sqrt, masked means, assemble [64, 64].
"""
import numpy as np

import concourse.bacc as bacc
import concourse.tile as tile
import concourse.mybir as mybir
from concourse.alu_op_type import AluOpType
from concourse.masks import make_identity
from concourse.bass_utils import run_bass_kernel_spmd

P = 128
N_CORES = 8
GN = GL = 64
D = 256
BIG = 1.0e4
F16 = mybir.dt.float16
F32 = mybir.dt.float32
I16 = mybir.dt.int16

_prog_cache = {}


def _wrap_idx(idx: np.ndarray) -> np.ndarray:
    """dma_gather index layout: [128, n/16] int16; idx i at [i%16, i//16], x8 groups."""
    n = idx.shape[0]
    base = np.ascontiguousarray(idx.reshape(n // 16, 16).T).astype(np.int16)
    return np.ascontiguousarray(np.tile(base, (8, 1)))


def _build(nrt: int, cols: int, n_h: int, lw: int):
    NG = cols // 2048          # psum column groups

    nc = bacc.Bacc("TRN2", target_bir_lowering=False, debug=False,
                   num_devices=N_CORES)
    hbf = nc.dram_tensor("hbf", [n_h, D], F16, kind="ExternalInput")
    idxn0 = nc.dram_tensor("idxn0", [P, nrt * 8], I16, kind="ExternalInput")
    idxn1 = nc.dram_tensor("idxn1", [P, nrt * 8], I16, kind="ExternalInput")
    ulT_in = nc.dram_tensor("ulT", [P, 2 * cols], F16, kind="ExternalInput")
    bl2_in = nc.dram_tensor("bl2", [1, cols], F16, kind="ExternalInput")
    maskq = nc.dram_tensor("maskq", [P, nrt], F32, kind="ExternalInput")
    rowout = nc.dram_tensor("rowout", [P, nrt * GL], F16, kind="ExternalOutput")
    collout = nc.dram_tensor("collout", [P, cols], F16, kind="ExternalOutput")

    with tile.TileContext(nc) as tc:
        with (
            tc.tile_pool(name="persist", bufs=1) as pp,
            tc.tile_pool(name="strip", bufs=2) as sp,
            tc.tile_pool(name="small", bufs=2) as smp,
        ):
            u_lT = pp.tile([P, 2, cols], F16, tag="u_lT")
            bl2v = pp.tile([1, cols], F16, tag="bl2v")
            u_nT = pp.tile([P, nrt * 2, P], F16, tag="u_nT")
            an2q = pp.tile([P, nrt], F32, tag="an2q")
            coll = pp.tile([P, cols], F16, tag="coll")
            ones_row = pp.tile([1, P], F16, tag="ones_row")
            ident = pp.tile([P, P], F16, tag="ident")
            it_n0 = pp.tile([P, nrt * 8], I16, tag="it_n0")
            it_n1 = pp.tile([P, nrt * 8], I16, tag="it_n1")
            maskt = pp.tile([P, nrt], F32, tag="maskt")
            gn0 = pp.tile([P, nrt, D], F16, tag="gn0")
            gn1 = pp.tile([P, nrt, D], F16, tag="gn1")
            u_n = pp.tile([P, nrt, D], F16, tag="u_n")

            nc.gpsimd.memset(ones_row[:], 1.0)
            make_identity(nc, ident[:])

            nc.sync.dma_start(u_lT[:].rearrange("p c n -> p (c n)"), ulT_in[:])
            nc.sync.dma_start(bl2v[:], bl2_in[:])
            nc.sync.dma_start(it_n0[:], idxn0[:])
            nc.sync.dma_start(it_n1[:], idxn1[:])
            nc.sync.dma_start(maskt[:], maskq[:])

            with tc.tile_pool(name="preps", bufs=2, space="PSUM") as preps:
                # ---- node side: gather + u, |u|^2, transpose ----
                nc.gpsimd.dma_gather(gn0[:], hbf[:], it_n0[:], nrt * P, nrt * P,
                                     D, single_packet=False)
                nc.gpsimd.dma_gather(gn1[:], hbf[:], it_n1[:], nrt * P, nrt * P,
                                     D, single_packet=False)
                nc.vector.tensor_add(u_n[:], gn0[:], gn1[:])
                for t in range(nrt):
                    scratch = smp.tile([P, D], F16, tag="scratch")
                    nc.vector.affine_mul_reduce(
                        out=scratch[:], accum_out=an2q[:, t:t + 1],
                        in0=u_n[:, t, :], in1=u_n[:, t, :],
                        scale=-0.25, bias=0.0,
                    )
                nc.vector.tensor_add(an2q[:], an2q[:], maskt[:])
                for t in range(nrt):
                    for k in range(2):
                        ptr = preps.tile([P, P], F16, tag="tr")
                        nc.tensor.transpose(
                            ptr[:], u_n[:, t, k * P:(k + 1) * P], ident[:])
                        nc.vector.tensor_scalar_mul(
                            u_nT[:, 2 * t + k, :], ptr[:], -2.0)

            # ---- main loop ----
            with tc.tile_pool(name="ps", bufs=2, space="PSUM") as ps:
                for t in range(nrt):
                    strip = sp.tile([P, cols], F16, tag="strip")
                    for g in range(NG):
                        pt = ps.tile([P, 2048], F32, tag="dot")
                        for s in range(4):
                            osl = slice(s * 512, (s + 1) * 512)
                            csl = slice(g * 2048 + s * 512, g * 2048 + (s + 1) * 512)
                            nc.tensor.matmul(pt[:, osl], u_nT[:, 2 * t, :],
                                             u_lT[:, 0, csl], start=True, stop=False)
                        for s in range(4):
                            osl = slice(s * 512, (s + 1) * 512)
                            csl = slice(g * 2048 + s * 512, g * 2048 + (s + 1) * 512)
                            nc.tensor.matmul(pt[:, osl], u_nT[:, 2 * t + 1, :],
                                             u_lT[:, 1, csl], start=False, stop=False)
                        for s in range(4):
                            osl = slice(s * 512, (s + 1) * 512)
                            csl = slice(g * 2048 + s * 512, g * 2048 + (s + 1) * 512)
                            nc.tensor.matmul(pt[:, osl], ones_row[:],
                                             bl2v[:, csl], start=False, stop=True)
                        nc.scalar.activation(
                            strip[:, g * 2048:(g + 1) * 2048], pt[:],
                            mybir.ActivationFunctionType.Identity,
                            bias=an2q[:, t:t + 1], scale=-0.25)
                    # row side: 5-way strided TT-max tree, then 3D reduce
                    tacc = smp.tile([P, GL, lw // 5], F16, tag="tacc")
                    v = strip[:].rearrange("p (s j w) -> p s j w", j=5, w=lw // 5)
                    nc.vector.tensor_max(tacc[:], v[:, :, 0, :], v[:, :, 1, :])
                    nc.vector.tensor_max(tacc[:], tacc[:], v[:, :, 2, :])
                    nc.vector.tensor_max(tacc[:], tacc[:], v[:, :, 3, :])
                    nc.vector.tensor_max(tacc[:], tacc[:], v[:, :, 4, :])
                    rmx = smp.tile([P, GL], F16, tag="rowmax")
                    nc.vector.tensor_reduce(
                        rmx[:], tacc[:], axis=mybir.AxisListType.X,
                        op=AluOpType.max)
                    nc.sync.dma_start(rowout[:, t * GL:(t + 1) * GL], rmx[:])
                    # col side: running per-lane max
                    if t == 0:
                        nc.vector.tensor_copy(coll[:], strip[:])
                    else:
                        nc.vector.tensor_max(coll[:], coll[:], strip[:])
                nc.sync.dma_start(collout[:], coll[:])

    nc.compile()
    return nc


def _get_program(nrt, cols, n_h, lw):
    key = (nrt, cols, n_h, lw)
    if key not in _prog_cache:
        _prog_cache[key] = _build(nrt, cols, n_h, lw)
    return _prog_cache[key]


def _band_layout(sizes, nrt):
    """Lane bands: segment s gets L_s = ceil(size_s/nrt) lanes."""
    L = [-(-int(s) // nrt) if s > 0 else 0 for s in sizes]
    B = np.concatenate([[0], np.cumsum(L)]).astype(np.int64)
    return B, L


def kernel(h, node_edge, node_batch, label_edge, label_batch):
    h = np.asarray(h)
    ne = np.asarray(node_edge).astype(np.int64)
    nb = np.asarray(node_batch).astype(np.int64)
    le = np.asarray(label_edge).astype(np.int64)
    lb = np.asarray(label_batch).astype(np.int64)
    n_h = h.shape[0]

    cn = np.bincount(nb, minlength=GN).astype(np.int64)
    cl = np.bincount(lb, minlength=GL).astype(np.int64)
    nb_off = np.concatenate([[0], np.cumsum(cn)])
    lb_off = np.concatenate([[0], np.cumsum(cl)])

    lw = 160
    while cl.max() > lw:
        lw += 32
    cols = GL * lw

    # nrt: minimal row-tile count s.t. per-core variable bands fit 128 lanes
    core_sizes = cn.reshape(N_CORES, 8)
    nrt = max(1, int(-(-core_sizes.sum(1).max() // P)))
    while max(sum(-(-int(s) // nrt) for s in core_sizes[c] if s > 0)
              for c in range(N_CORES)) > P:
        nrt += 1

    nrows = nrt * P

    # label columns (shared): pad each segment to lw with duplicate edges
    k_in_seg = np.tile(np.arange(lw), GL)
    seg_of_col = np.repeat(np.arange(GL), lw)
    sz = cl[seg_of_col]
    col_w = (k_in_seg < sz).astype(np.float64)
    col_edge = np.where(sz > 0, lb_off[seg_of_col] + (k_in_seg % np.maximum(sz, 1)), 0)

    hf = h.astype(np.float32)
    u_l = hf[le[0][col_edge]] + hf[le[1][col_edge]]            # [cols, 256]
    bl2 = (u_l * u_l).sum(axis=1)                              # [cols]
    ulT = np.ascontiguousarray(
        u_l.T.astype(np.float16).reshape(2, P, cols).transpose(1, 0, 2)
        .reshape(P, 2 * cols))
    bl2_f16 = np.ascontiguousarray(bl2.astype(np.float16).reshape(1, cols))
    hf16 = np.ascontiguousarray(h.astype(np.float16))

    in_maps = []
    band_info = []
    for c in range(N_CORES):
        B, L = _band_layout(core_sizes[c], nrt)
        assert B[-1] <= P
        slot = np.full(nrows, -1, np.int64)
        for s in range(8):
            g = 8 * c + s
            n_g = int(cn[g])
            if n_g == 0:
                continue
            j = np.arange(n_g)
            lanes = B[s] + j // nrt
            ts = j % nrt
            slot[ts * P + lanes] = nb_off[g] + j
        rows_safe = np.where(slot < 0, 0, slot)
        mq = np.where(slot < 0, -BIG, 0.0).astype(np.float32)
        in_maps.append({
            "hbf": hf16,
            "idxn0": _wrap_idx(ne[0][rows_safe].astype(np.int16)),
            "idxn1": _wrap_idx(ne[1][rows_safe].astype(np.int16)),
            "ulT": ulT,
            "bl2": bl2_f16,
            "maskq": np.ascontiguousarray(mq.reshape(nrt, P).T),
        })
        band_info.append((B, L))

    nc = _get_program(nrt, cols, n_h, lw)
    res = run_bass_kernel_spmd(nc, in_maps, core_ids=list(range(N_CORES)))

    out_n = np.zeros((GN, GL), np.float64)
    out_l = np.zeros((GN, GL), np.float64)
    for c in range(N_CORES):
        r = res.results[c]
        rowe = r["rowout"].astype(np.float64).reshape(P, nrt, GL)
        colle = r["collout"].astype(np.float64)                 # [128, cols]
        B, L = band_info[c]
        for s in range(8):
            g = 8 * c + s
            n_g = int(cn[g])
            if n_g == 0:
                continue
            j = np.arange(n_g)
            lanes = B[s] + j // nrt
            ts = j % nrt
            ev = rowe[lanes, ts, :]                             # [seg_rows, GL]
            dmin = np.sqrt(np.maximum(-ev, 0.0))
            row_mean = -dmin.mean(axis=0)
            row_mean[cl == 0] = 0.0
            out_n[g] = row_mean

            ecol = colle[B[s]:B[s] + L[s], :].max(axis=0)       # [cols]
            dcol = np.sqrt(np.maximum(-ecol, 0.0))
            sums = (dcol * col_w).reshape(GL, lw).sum(axis=1)
            col_mean = -(sums / np.maximum(cl, 1))
            col_mean[cl == 0] = 0.0
            out_l[g] = col_mean

    return ((out_n + out_l) * 0.5).astype(np.float32)


# revision 11
# speedup vs baseline: 2.5113x; 1.0232x over previous
"""Trainium2 Bass kernel for nn_Explainer segment_reduce (cdist + bidirectional
segment max/mean) on 8 NeuronCores.

Math (reference):
    ef_n = (h[ne0] + h[ne1])/2, ef_l = (h[le0] + h[le1])/2
    M = -cdist(ef_n, ef_l)                      # [En, El]
    out_n = seg_mean_rows(seg_max_cols(M))      # [Gn, Gl]
    out_l = seg_mean_cols(seg_max_rows(M))      # [Gn, Gl]
    out = (out_n + out_l)/2

Sharding: core c owns node segments [8c, 8c+8) (data parallel over node
edges); label-edge features are replicated (host-computed u_l = h[le0]+h[le1],
padded per label segment to LW columns with duplicated edges; dups can't
change a segment min and are weight-masked out of the segment mean).

Per core, with u = h[e0]+h[e1] (ef = u/2): d^2 = 0.25*(|u_n|^2+|u_l|^2-2u_n.u_l).
  - Node rows live in per-segment lane bands: segment s -> lanes
    [B_s, B_s+L_s), row-tiles t in [0, nrt). Dummy slots get -BIG bias.
  - PE: psum = |u_l|^2 - 2 u_n.u_l via K=257 matmuls (2 K=128 chunks of
    -2*u_nT gathered+transposed on device, plus a K=1 ones x bl2 row).
  - ACT: e = -(0.25*psum) - 0.25*|u_n|^2 (- BIG if dummy) -> fp16 strip.
    e == -d^2 exactly; negation turns all mins into maxes.
  - DVE row side: 5-way strided TT-max tree + 3D reduce -> e_rowmax [128,64].
  - DVE col side: running TT-max into coll [128, cols].
Host: band-collapse coll, clamp/sqrt, masked means, assemble [64, 64].
"""
import numpy as np

import concourse.bacc as bacc
import concourse.tile as tile
import concourse.mybir as mybir
from concourse.alu_op_type import AluOpType
from concourse.masks import make_identity
from concourse.bass_utils import run_bass_kernel_spmd

P = 128
N_CORES = 8
GN = GL = 64
D = 256
BIG = 1.0e4
F16 = mybir.dt.float16
F32 = mybir.dt.float32
I16 = mybir.dt.int16

_prog_cache = {}


def _wrap_idx(idx: np.ndarray) -> np.ndarray:
    """dma_gather index layout: [128, n/16] int16; idx i at [i%16, i//16], x8 groups."""
    n = idx.shape[0]
    base = np.ascontiguousarray(idx.reshape(n // 16, 16).T).astype(np.int16)
    return np.ascontiguousarray(np.tile(base, (8, 1)))


def _build(nrt: int, cols: int, n_h: int, lw: int):
    NG = cols // 2048          # psum column groups

    nc = bacc.Bacc("TRN2", target_bir_lowering=False, debug=False,
                   num_devices=N_CORES)
    hbf = nc.dram_tensor("hbf", [n_h, D], F16, kind="ExternalInput")
    idxn0 = nc.dram_tensor("idxn0", [P, nrt * 8], I16, kind="ExternalInput")
    idxn1 = nc.dram_tensor("idxn1", [P, nrt * 8], I16, kind="ExternalInput")
    ulT_in = nc.dram_tensor("ulT", [P, 2 * cols], F16, kind="ExternalInput")
    bl2_in = nc.dram_tensor("bl2r", [P, cols], F16, kind="ExternalInput")
    maskq = nc.dram_tensor("maskq", [P, nrt], F32, kind="ExternalInput")
    rowout = nc.dram_tensor("rowout", [P, nrt * GL], F16, kind="ExternalOutput")
    collout = nc.dram_tensor("collout", [P, cols], F16, kind="ExternalOutput")

    with tile.TileContext(nc) as tc:
        with (
            tc.tile_pool(name="persist", bufs=1) as pp,
            tc.tile_pool(name="strip", bufs=2) as sp,
            tc.tile_pool(name="small", bufs=2) as smp,
        ):
            u_lT = pp.tile([P, 2, cols], F16, tag="u_lT")
            bl2rep = pp.tile([P, cols], F16, tag="bl2rep")
            u_nT = pp.tile([P, nrt * 2, P], F16, tag="u_nT")
            an2q = pp.tile([P, nrt], F32, tag="an2q")
            coll = pp.tile([P, cols], F16, tag="coll")
            onesrep = pp.tile([P, P], F16, tag="onesrep")
            ident = pp.tile([P, P], F16, tag="ident")
            it_n0 = pp.tile([P, nrt * 8], I16, tag="it_n0")
            it_n1 = pp.tile([P, nrt * 8], I16, tag="it_n1")
            maskt = pp.tile([P, nrt], F32, tag="maskt")
            gn0 = pp.tile([P, nrt, D], F16, tag="gn0")
            gn1 = pp.tile([P, nrt, D], F16, tag="gn1")
            u_n = pp.tile([P, nrt, D], F16, tag="u_n")

            nc.gpsimd.memset(onesrep[:], 1.0)
            make_identity(nc, ident[:])

            # chunked u_lT loads so the first matmuls can start early
            for k in range(2):
                for g in range(NG):
                    nc.sync.dma_start(
                        u_lT[:, k, g * 2048:(g + 1) * 2048],
                        ulT_in[:, k * cols + g * 2048:k * cols + (g + 1) * 2048])
            nc.sync.dma_start(bl2rep[:], bl2_in[:])
            nc.sync.dma_start(it_n0[:], idxn0[:])
            nc.sync.dma_start(it_n1[:], idxn1[:])
            nc.sync.dma_start(maskt[:], maskq[:])

            with tc.tile_pool(name="preps", bufs=2, space="PSUM") as preps:
                # ---- node side, pipelined per row-tile ----
                for t in range(nrt):
                    nc.gpsimd.dma_gather(gn0[:, t:t + 1, :], hbf[:],
                                         it_n0[:, t * 8:(t + 1) * 8], P, P, D)
                    nc.gpsimd.dma_gather(gn1[:, t:t + 1, :], hbf[:],
                                         it_n1[:, t * 8:(t + 1) * 8], P, P, D)
                    nc.vector.tensor_add(u_n[:, t, :], gn0[:, t, :], gn1[:, t, :])
                    scratch = smp.tile([P, D], F16, tag="scratch")
                    nc.vector.affine_mul_reduce(
                        out=scratch[:], accum_out=an2q[:, t:t + 1],
                        in0=u_n[:, t, :], in1=u_n[:, t, :],
                        scale=-0.25, bias=0.0,
                    )
                    nc.vector.tensor_add(an2q[:, t:t + 1], an2q[:, t:t + 1],
                                         maskt[:, t:t + 1])
                    for k in range(2):
                        ptr = preps.tile([P, P], F16, tag="tr")
                        nc.tensor.transpose(
                            ptr[:], u_n[:, t, k * P:(k + 1) * P], ident[:])
                        nc.vector.tensor_scalar_mul(
                            u_nT[:, 2 * t + k, :], ptr[:], -2.0)

            # ---- main loop ----
            with tc.tile_pool(name="ps", bufs=2, space="PSUM") as ps:
                for t in range(nrt):
                    strip = sp.tile([P, cols], F16, tag="strip")
                    for g in range(NG):
                        pt = ps.tile([P, 2048], F32, tag="dot")
                        for s in range(4):
                            osl = slice(s * 512, (s + 1) * 512)
                            csl = slice(g * 2048 + s * 512, g * 2048 + (s + 1) * 512)
                            nc.tensor.matmul(pt[:, osl], u_nT[:, 2 * t, :],
                                             u_lT[:, 0, csl], start=True, stop=False)
                        for s in range(4):
                            osl = slice(s * 512, (s + 1) * 512)
                            csl = slice(g * 2048 + s * 512, g * 2048 + (s + 1) * 512)
                            nc.tensor.matmul(pt[:, osl], u_nT[:, 2 * t + 1, :],
                                             u_lT[:, 1, csl], start=False, stop=False)
                        for s in range(4):
                            osl = slice(s * 512, (s + 1) * 512)
                            csl = slice(g * 2048 + s * 512, g * 2048 + (s + 1) * 512)
                            pb = 32 * s
                            nc.tensor.matmul(pt[:, osl], onesrep[pb:pb + 1, :],
                                             bl2rep[pb:pb + 1, csl],
                                             start=False, stop=True,
                                             tile_position=(pb, 0))
                        nc.scalar.activation(
                            strip[:, g * 2048:(g + 1) * 2048], pt[:],
                            mybir.ActivationFunctionType.Identity,
                            bias=an2q[:, t:t + 1], scale=-0.25)
                    # row side: 5-way strided TT-max tree, then 3D reduce
                    tacc = smp.tile([P, GL, lw // 5], F16, tag="tacc")
                    v = strip[:].rearrange("p (s j w) -> p s j w", j=5, w=lw // 5)
                    nc.vector.tensor_max(tacc[:], v[:, :, 0, :], v[:, :, 1, :])
                    nc.vector.tensor_max(tacc[:], tacc[:], v[:, :, 2, :])
                    nc.vector.tensor_max(tacc[:], tacc[:], v[:, :, 3, :])
                    nc.vector.tensor_max(tacc[:], tacc[:], v[:, :, 4, :])
                    rmx = smp.tile([P, GL], F16, tag="rowmax")
                    nc.vector.tensor_reduce(
                        rmx[:], tacc[:], axis=mybir.AxisListType.X,
                        op=AluOpType.max)
                    nc.sync.dma_start(rowout[:, t * GL:(t + 1) * GL], rmx[:])
                    # col side: running per-lane max
                    if t == 0:
                        nc.vector.tensor_copy(coll[:], strip[:])
                    elif t < nrt - 1:
                        nc.vector.tensor_max(coll[:], coll[:], strip[:])
                    else:
                        for g in range(NG):
                            gsl = slice(g * 2048, (g + 1) * 2048)
                            nc.vector.tensor_max(coll[:, gsl], coll[:, gsl],
                                                 strip[:, gsl])
                            nc.sync.dma_start(collout[:, gsl], coll[:, gsl])

    nc.compile()
    return nc


def _get_program(nrt, cols, n_h, lw):
    key = (nrt, cols, n_h, lw)
    if key not in _prog_cache:
        _prog_cache[key] = _build(nrt, cols, n_h, lw)
    return _prog_cache[key]


def _band_layout(sizes, nrt):
    """Lane bands: segment s gets L_s = ceil(size_s/nrt) lanes."""
    L = [-(-int(s) // nrt) if s > 0 else 0 for s in sizes]
    B = np.concatenate([[0], np.cumsum(L)]).astype(np.int64)
    return B, L


def kernel(h, node_edge, node_batch, label_edge, label_batch):
    h = np.asarray(h)
    ne = np.asarray(node_edge).astype(np.int64)
    nb = np.asarray(node_batch).astype(np.int64)
    le = np.asarray(label_edge).astype(np.int64)
    lb = np.asarray(label_batch).astype(np.int64)
    n_h = h.shape[0]

    cn = np.bincount(nb, minlength=GN).astype(np.int64)
    cl = np.bincount(lb, minlength=GL).astype(np.int64)
    nb_off = np.concatenate([[0], np.cumsum(cn)])
    lb_off = np.concatenate([[0], np.cumsum(cl)])

    lw = 160
    while cl.max() > lw:
        lw += 32
    cols = GL * lw

    # nrt: minimal row-tile count s.t. per-core variable bands fit 128 lanes
    core_sizes = cn.reshape(N_CORES, 8)
    nrt = max(1, int(-(-core_sizes.sum(1).max() // P)))
    while max(sum(-(-int(s) // nrt) for s in core_sizes[c] if s > 0)
              for c in range(N_CORES)) > P:
        nrt += 1

    nrows = nrt * P

    # label columns (shared): pad each segment to lw with duplicate edges
    k_in_seg = np.tile(np.arange(lw), GL)
    seg_of_col = np.repeat(np.arange(GL), lw)
    sz = cl[seg_of_col]
    col_w = (k_in_seg < sz).astype(np.float64)
    col_edge = np.where(sz > 0, lb_off[seg_of_col] + (k_in_seg % np.maximum(sz, 1)), 0)

    hf = h.astype(np.float32)
    u_l = hf[le[0][col_edge]] + hf[le[1][col_edge]]            # [cols, 256]
    bl2 = (u_l * u_l).sum(axis=1)                              # [cols]
    ulT = np.ascontiguousarray(
        u_l.T.astype(np.float16).reshape(2, P, cols).transpose(1, 0, 2)
        .reshape(P, 2 * cols))
    bl2_f16 = np.ascontiguousarray(
        np.broadcast_to(bl2.astype(np.float16).reshape(1, cols), (P, cols)))
    hf16 = np.ascontiguousarray(h.astype(np.float16))

    in_maps = []
    band_info = []
    for c in range(N_CORES):
        B, L = _band_layout(core_sizes[c], nrt)
        assert B[-1] <= P
        slot = np.full(nrows, -1, np.int64)
        for s in range(8):
            g = 8 * c + s
            n_g = int(cn[g])
            if n_g == 0:
                continue
            j = np.arange(n_g)
            lanes = B[s] + j // nrt
            ts = j % nrt
            slot[ts * P + lanes] = nb_off[g] + j
        rows_safe = np.where(slot < 0, 0, slot)
        mq = np.where(slot < 0, -BIG, 0.0).astype(np.float32)
        in_maps.append({
            "hbf": hf16,
            "idxn0": _wrap_idx(ne[0][rows_safe].astype(np.int16)),
            "idxn1": _wrap_idx(ne[1][rows_safe].astype(np.int16)),
            "ulT": ulT,
            "bl2r": bl2_f16,
            "maskq": np.ascontiguousarray(mq.reshape(nrt, P).T),
        })
        band_info.append((B, L))

    nc = _get_program(nrt, cols, n_h, lw)
    res = run_bass_kernel_spmd(nc, in_maps, core_ids=list(range(N_CORES)))

    out_n = np.zeros((GN, GL), np.float64)
    out_l = np.zeros((GN, GL), np.float64)
    for c in range(N_CORES):
        r = res.results[c]
        rowe = r["rowout"].astype(np.float64).reshape(P, nrt, GL)
        colle = r["collout"].astype(np.float64)                 # [128, cols]
        B, L = band_info[c]
        for s in range(8):
            g = 8 * c + s
            n_g = int(cn[g])
            if n_g == 0:
                continue
            j = np.arange(n_g)
            lanes = B[s] + j // nrt
            ts = j % nrt
            ev = rowe[lanes, ts, :]                             # [seg_rows, GL]
            dmin = np.sqrt(np.maximum(-ev, 0.0))
            row_mean = -dmin.mean(axis=0)
            row_mean[cl == 0] = 0.0
            out_n[g] = row_mean

            ecol = colle[B[s]:B[s] + L[s], :].max(axis=0)       # [cols]
            dcol = np.sqrt(np.maximum(-ecol, 0.0))
            sums = (dcol * col_w).reshape(GL, lw).sum(axis=1)
            col_mean = -(sums / np.maximum(cl, 1))
            col_mean[cl == 0] = 0.0
            out_l[g] = col_mean

    return ((out_n + out_l) * 0.5).astype(np.float32)
